# revision 1
# baseline (speedup 1.0000x reference)
"""AuthorGroupAttention Trainium2 kernel.

Data-parallel over batch: 8 samples -> 8 NeuronCores, one sample per core.
Routing resolved on host (per-core reader-group weights gathered, cast and
laid out per-engine-friendly in _host_prep).

Precision (validated vs reference: rel err ~5e-3 against the 2e-2 gate):
  - generic path: fp16 matmul operands everywhere, fp32 PSUM accumulation
  - reader path (weight 0.1 in the prob combine): fp8e4 operands with
    DoubleRow matmuls (0.5 cyc/row); its exp is a Schraudolph bit-trick on
    DVE (scores*1/ln2 + 55.2 rounded to uint8 = e4m3 bits of exp(scores/8)),
    consistent numerator/denominator so the approximation bias cancels in
    the softmax normalization.

Structure:
  - q/k produced in [d, t] layout (d on partitions): per head-pair (gen,
    fp16) / head-quad (rdr, fp8 DR with 32-row subtiles at tile_position)
    projection chains, interleaved into the attention loop as PE filler
    ("pump") with per-boundary forced drains.
  - scores per (head, s-block): gen [128,1024] psum tiles (2-deep pool),
    rdr th-split [128,512] tiles in a shared 4-deep "small" pool that also
    carries av accumulators, projection chains, and deferred v-proj.
  - attention transposed: stationary = exp tile [s,t], moving = [v | 1/w]
    so attn lands [t, d] with the softmax denominator Z/w in the extra
    column; one ACT copy bounces the accumulator to SBUF, gpsimd
    normalize_recip applies w/Z per path, gpsimd add combines gen+rdr.
  - attn [t, e] tiles are DMA-transposed (xbar) per (pair, t-block) into
    [e, t] for the fp16 output projection; v-bias is folded into the output
    bias on host (combined prob rows sum to 0.5).
  - AV/combine of head h-1 is software-pipelined into head h's score loop;
    v-projection for pairs 6-7 is deferred into the pump as late filler.
"""

import os
import sys

for _p in ("/opt/trn_rl_repo",):
    if os.path.isdir(_p) and _p not in sys.path:
        sys.path.insert(0, _p)

import numpy as np

import concourse.bass as bass
import concourse.mybir as mybir
from concourse import bacc
from concourse.tile import TileContext
from concourse.bass_utils import run_bass_kernel_spmd

B, T, E, H, G = 8, 1024, 1024, 16, 4
D = E // H  # 64
SCALING = float(D) ** -0.5
W_G = 0.9 / 2.0
W_R = 0.1 / 2.0
EO = 8
SO = 8
TB = 8
NP = 8  # head pairs
NQ = 4  # head quads

F32 = mybir.dt.float32
F16 = mybir.dt.float16
F8 = mybir.dt.float8e4
U8 = mybir.dt.uint8
DRM = mybir.MatmulPerfMode.DoubleRow
EXP = mybir.ActivationFunctionType.Exp
MULT = mybir.AluOpType.mult
ADD = mybir.AluOpType.add

# Schraudolph constants for e4m3 bits: bits = round(score*A + Bc)
SCH_A = SCALING * 8.0 / float(np.log(2.0))
SCH_B = 56.0 - 0.8

USE_RDR_SCORE_DR = True  # DoubleRow with 32-row subtiles for reader scores


def build_nc():
    nc = bacc.Bacc(name="author_group_attention_v2")

    hsT16 = nc.dram_tensor("hsT16", [E, T], F16, kind="ExternalInput")
    hsT8 = nc.dram_tensor("hsT8", [E, T], F8, kind="ExternalInput")
    wg = nc.dram_tensor("wg", [NP, 128, 2, EO, 128], F16, kind="ExternalInput")
    w8 = nc.dram_tensor("w8", [NQ, 128, 2, 2, EO, 128], F8, kind="ExternalInput")
    wv = nc.dram_tensor("wv", [128, EO, E], F16, kind="ExternalInput")
    wo = nc.dram_tensor("wo", [TB, 128, NP, 128], F16, kind="ExternalInput")
    gbias = nc.dram_tensor("gbias", [128, 2, NP], F32, kind="ExternalInput")
    rbias = nc.dram_tensor("rbias", [128, 2, NQ, 2], F32, kind="ExternalInput")
    bo = nc.dram_tensor("bo", [128, TB], F32, kind="ExternalInput")
    outT = nc.dram_tensor("outT", [E, T], F16, kind="ExternalOutput")

    hsT16_r = hsT16.rearrange("(eo p) t -> p eo t", p=128)
    hsT8_r = hsT8.rearrange("(eo p) t -> p eo t", p=128)


    with TileContext(nc) as tc:
        from contextlib import ExitStack

        with ExitStack() as stack:
            const = stack.enter_context(tc.tile_pool(name="const", bufs=1))

            hsT16_sb = const.tile([128, EO, T], F16, tag="hsT16")
            hs8_sb = const.tile([128, EO, T], F8, tag="hs8")
            v16_sb = const.tile([128, SO, H, 66], F16, tag="v16")
            wv_sb = const.tile([128, EO, E], F16, tag="wv")
            v8_sb = const.tile([128, SO, H, 66], F8, tag="v8")
            gbias_sb = const.tile([128, 2, NP], F32, tag="gbias")
            rbias_sb = const.tile([128, 2, NQ, 2], F32, tag="rbias")
            bo_sb = const.tile([128, TB], F32, tag="bo")
            comb_tiles = [
                const.tile([128, E], F16, tag=f"comb{tb}", name=f"comb{tb}")
                for tb in range(TB)
            ]
            attnT_tiles = [
                const.tile([128, T], F16, tag=f"attnT{p}", name=f"attnT{p}")
                for p in range(NP)
            ]

            # ones columns pre-scaled by 1/w so reciprocal gives w/Z
            nc.vector.memset(v16_sb[:, :, :, 64:65], 1.0 / W_G)
            nc.vector.memset(v8_sb[:, :, :, 64:65], 1.0 / W_R)


            # persistent pools used across prologue + main
            qk16p = stack.enter_context(tc.tile_pool(name="qk16", bufs=2))
            qk8p = stack.enter_context(tc.tile_pool(name="qk8", bufs=2))
            wgp = stack.enter_context(tc.tile_pool(name="wgp", bufs=2))
            w8p = stack.enter_context(tc.tile_pool(name="w8p", bufs=2))

            def gen_proj_steps(pair, pp2_pool):
                """Generic q/k projection for a head pair -> list of step
                closures (PE matmuls + ACT drains). Weight DMAs fire now."""
                steps = []
                outs = {}
                wt2 = wgp.tile([128, 2, EO, 128], F16, tag="wg")
                nc.sync.dma_start(wt2[:], wg[pair])
                for qk in range(2):
                    wt = wt2[:, qk]
                    dst = qk16p.tile([128, T], F16, tag=f"qk16_{qk}")
                    outs[qk] = dst
                    state = {}

                    def mk(qk, wt, dst, state, th, eo):
                        def step():
                            if eo == 0:
                                state[th] = pp2_pool.tile(
                                    [128, 512], F32, tag="sm", name=f"gp{pair}{qk}{th}"
                                )
                            nc.tensor.matmul(
                                state[th][:],
                                wt[:, eo, :],
                                hsT16_sb[:, eo, th * 512 : (th + 1) * 512],
                                start=(eo == 0),
                                stop=(eo == EO - 1),
                            )
                            if eo == EO - 1:
                                if th == 0:
                                    nc.scalar.activation(
                                        dst[:, 0:512],
                                        state[th][:],
                                        mybir.ActivationFunctionType.Identity,
                                        bias=gbias_sb[:, qk, pair : pair + 1],
                                    )
                                else:
                                    nc.vector.tensor_scalar_add(
                                        dst[:, 512:1024],
                                        state[th][:],
                                        gbias_sb[:, qk, pair : pair + 1],
                                    )
                        return step

                    for th in range(2):
                        for eo in range(EO):
                            steps.append(mk(qk, wt, dst, state, th, eo))
                return outs, steps

            def vproj_steps(pair, pool, tag):
                """One closure per s-block: a full 8-matmul chain + drains,
                so each pump pop contributes ~0.4us of PE work instead of
                a single 53ns micro-matmul."""
                steps = []
                ocols = slice(pair * 128, (pair + 1) * 128)

                def mk(sb):
                    def step():
                        pv = pool.tile(
                            [128, 512], F32, tag=tag, name=f"pv{pair}_{sb}"
                        )
                        for eo in range(EO):
                            nc.tensor.matmul(
                                pv[:, 0:128],
                                hsT16_sb[:, eo, sb * 128 : (sb + 1) * 128],
                                wv_sb[:, eo, ocols],
                                start=(eo == 0),
                                stop=(eo == EO - 1),
                            )
                        pv_r = pv[:, 0:128].rearrange(
                            "p (hh dd) -> p hh dd", dd=64
                        )
                        vsl = v16_sb[:, sb, 2 * pair : 2 * pair + 2, 0:64]
                        nc.vector.tensor_copy(vsl, pv_r)
                        nc.gpsimd.tensor_copy(
                            v8_sb[:, sb, 2 * pair : 2 * pair + 2, 0:64], vsl
                        )
                    return step

                for sb in range(SO):
                    steps.append(mk(sb))
                return steps

            def rdr_proj_steps(quad, pp2_pool):
                """Reader q/k projection for a head quad (fp8 DoubleRow).
                Outputs supertiles [128(=4h x 32d), 2(d-half), T] fp8."""
                steps = []
                outs = {}
                wt4 = w8p.tile([128, 2, 2, EO, 128], F8, tag="w8")
                nc.sync.dma_start(wt4[:], w8[quad])
                for qk in range(2):
                    dst = qk8p.tile([128, 2, T], F8, tag=f"qk8_{qk}")
                    outs[qk] = dst
                    for ab in range(2):
                        wt = wt4[:, qk, ab]
                        state = {}

                        def mk(qk, ab, wt, dst, th):
                            def step():
                                po = pp2_pool.tile(
                                    [128, 512], F32, tag="sm",
                                    name=f"rp{quad}{qk}{ab}{th}",
                                )
                                for a in range(4):
                                    nc.tensor.matmul(
                                        po[:],
                                        wt[:, 2 * a : 2 * a + 2, :],
                                        hs8_sb[:, 2 * a : 2 * a + 2,
                                               th * 512 : (th + 1) * 512],
                                        start=(a == 0),
                                        stop=(a == 3),
                                        perf_mode=DRM,
                                    )
                                if th == 0:
                                    nc.vector.tensor_scalar_add(
                                        dst[:, ab, 0:512], po[:],
                                        rbias_sb[:, qk, quad, ab : ab + 1],
                                    )
                                else:
                                    nc.scalar.activation(
                                        dst[:, ab, 512:1024], po[:],
                                        mybir.ActivationFunctionType.Identity,
                                        bias=rbias_sb[:, qk, quad, ab : ab + 1],
                                    )
                            return step

                        for th in range(2):
                            steps.append(mk(qk, ab, wt, dst, th))
                return outs, steps

            # ---------------- prologue: v proj + first projections ----------
            pump = []
            with tc.tile_pool(
                name="vps", bufs=2, space="PSUM"
            ) as vps, tc.tile_pool(name="pps", bufs=2, space="PSUM") as pps:
                # first chunk pair ahead of the 0.77MB weight DMAs so the
                # v-chains start immediately; weights next (prologue pump)
                nc.sync.dma_start(hsT16_sb[:, 0:1], hsT16_r[:, 0:1])
                nc.sync.dma_start(wv_sb[:, 0:1, 0:512], wv[:, 0:1, 0:512])
                qk0, steps_g0 = gen_proj_steps(0, pps)
                r0, steps_r0 = rdr_proj_steps(0, pps)
                for a, b in ((1, 2), (2, 4), (4, 6), (6, 8)):
                    nc.sync.dma_start(hsT16_sb[:, a:b], hsT16_r[:, a:b])
                    nc.sync.dma_start(wv_sb[:, a:b, 0:512], wv[:, a:b, 0:512])
                    if a == 2:
                        nc.sync.dma_start(gbias_sb[:], gbias[:])
                        nc.sync.dma_start(rbias_sb[:], rbias[:])
                        nc.sync.dma_start(bo_sb[:], bo[:])
                for a, b in ((0, 4), (4, 8)):
                    nc.sync.dma_start(hs8_sb[:, a:b], hsT8_r[:, a:b])
                # wv columns for the deferred v-proj pairs 6-7 (needed h>=9)
                nc.sync.dma_start(wv_sb[:, :, 512:1024], wv[:, :, 512:1024])
                pro_pump = steps_g0 + steps_r0

                for sb in range(SO):
                    pv = vps.tile([128, E], F32, tag="pv", name=f"pv{sb}")
                    for eo in range(EO):
                        nc.tensor.matmul(
                            pv[:, 0:512],
                            hsT16_sb[:, eo, sb * 128 : (sb + 1) * 128],
                            wv_sb[:, eo, 0:512],
                            start=(eo == 0),
                            stop=(eo == EO - 1),
                        )
                        for _ in range(2 if sb >= 4 else 1):
                            if pro_pump:
                                pro_pump.pop(0)()
                    pv_r = pv[:, 0:512].rearrange("p (hh dd) -> p hh dd", dd=64)
                    nc.scalar.copy(v16_sb[:, sb, 0:8, 0:64], pv_r)
                    nc.vector.tensor_copy(v8_sb[:, sb, 0:8, 0:64], pv_r)
                while pro_pump:
                    pro_pump.pop(0)()

            # ---------------- main attention loop ---------------------------
            # Software pipeline: the AV/combine work of head h-1 is emitted
            # interleaved into the scores/exp loop of head h, so the PE has
            # filler work while ACT/DVE drain the score tiles.
            wo_sb = const.tile([128, TB, NP, 128], F16, tag="wo_all")
            for j in range(TB):
                nc.sync.dma_start(wo_sb[:, j], wo[j])

            with ExitStack() as mstack:
                scp = mstack.enter_context(
                    tc.tile_pool(name="scp", bufs=2, space="PSUM")
                )
                smallp = mstack.enter_context(
                    tc.tile_pool(name="smallp", bufs=4, space="PSUM")
                )
                ex16p = mstack.enter_context(tc.tile_pool(name="ex16", bufs=2))
                ex8p = mstack.enter_context(tc.tile_pool(name="ex8", bufs=2))
                tmpp = mstack.enter_context(tc.tile_pool(name="tmpp", bufs=8))
                avsp = mstack.enter_context(tc.tile_pool(name="avsp", bufs=6))

                def av_steps(h, ex16, ex8):
                    """AV + combine for head h as a list of step closures.
                    Each tb yields 3 steps: gen-av mms, rdr-av mms, combine."""
                    pair, hp = h // 2, h % 2
                    vg = v16_sb[:, :, h, :]
                    v8h = v8_sb[:, :, h, :]
                    steps = []
                    state = {}

                    def mk_gen(tb):
                        def step():
                            tsl = slice(tb * 128, (tb + 1) * 128)
                            av = smallp.tile([128, 512], F32, tag="sm",
                                             name=f"av{h}_{tb}")
                            state[tb] = av
                            for a in range(SO):
                                nc.tensor.matmul(
                                    av[:, 0:65],
                                    ex16[:, a, tsl],
                                    vg[:, a, 0:65],
                                    start=(a == 0),
                                    stop=(a == SO - 1),
                                )
                        return step

                    def mk_rdr(tb):
                        def step():
                            tsl = slice(tb * 128, (tb + 1) * 128)
                            av = state[tb]
                            for a in range(4):
                                nc.tensor.matmul(
                                    av[:, 68:133],
                                    ex8[:, 2 * a : 2 * a + 2, tsl],
                                    v8h[:, 2 * a : 2 * a + 2, 0:65],
                                    start=(a == 0),
                                    stop=(a == 3),
                                    perf_mode=DRM,
                                )
                        return step

                    def mk_comb(tb):
                        def step():
                            av = state.pop(tb)
                            csl = slice(h * 64, h * 64 + 64)
                            tmp = tmpp.tile([128, 64], F16, tag="tmp")
                            avs = avsp.tile([128, 133], F32, tag="avs")
                            nc.scalar.copy(avs[:], av[:, 0:133])
                            nc.gpsimd.normalize_recip(
                                comb_tiles[tb][:, csl], avs[:, 0:64],
                                avs[:, 64:65],
                            )
                            nc.gpsimd.normalize_recip(
                                tmp[:], avs[:, 68:132], avs[:, 132:133],
                            )
                            nc.gpsimd.tensor_tensor(
                                comb_tiles[tb][:, csl],
                                comb_tiles[tb][:, csl],
                                tmp[:],
                                ADD,
                            )
                            if hp == 1:
                                nc.sync.dma_start_transpose(
                                    attnT_tiles[pair][
                                        :, tb * 128 : (tb + 1) * 128
                                    ],
                                    comb_tiles[tb][
                                        :, pair * 128 : (pair + 1) * 128
                                    ],
                                )
                        return step

                    for tb in range(TB):
                        steps.append(mk_gen(tb))
                        steps.append(mk_rdr(tb))
                        steps.append(mk_comb(tb))
                    return steps

                Qg = Kg = Q8 = K8 = None
                av_q = []  # pending av steps of the previous head

                def fill(n):
                    """Emit up to n units of filler: av steps take priority
                    (they unblock comb tiles), then proj pump steps."""
                    for _ in range(n):
                        if av_q:
                            av_q.pop(0)()
                        elif pump:
                            pump.pop(0)()

                for h in range(H):
                    pair, quad = h // 2, h // 4
                    hp, hq = h % 2, h % 4
                    if h == 0:
                        Qg, Kg = qk0[0], qk0[1]
                        Q8, K8 = r0[0], r0[1]
                        nxt_g = nxt_r = None
                    if h in (7, 9, 11, 13):
                        pump.extend(vproj_steps((h - 7) // 2 + 4, smallp, "sm"))
                    if hp == 0 and pair + 1 < NP:
                        nxt_g, s = gen_proj_steps(pair + 1, smallp)
                        pump.extend(s)
                    if hq == 0 and quad + 1 < NQ:
                        nxt_r, s = rdr_proj_steps(quad + 1, smallp)
                        pump.extend(s)

                    ex16 = ex16p.tile([128, SO, T], F16, tag="ex16")
                    ex8 = ex8p.tile([128, SO, T], F8, tag="ex8")

                    grow = slice(64 * hp, 64 * hp + 64)
                    rrow = slice(32 * hq, 32 * hq + 32)

                    for sb in range(SO):
                        ssl = slice(sb * 128, (sb + 1) * 128)
                        sc = scp.tile([128, T], F32, tag="sc", name=f"sc{h}_{sb}")
                        for th in range(2):
                            nc.tensor.matmul(
                                sc[:, th * 512 : (th + 1) * 512],
                                Kg[grow, ssl],
                                Qg[grow, th * 512 : (th + 1) * 512],
                                start=True, stop=True,
                            )
                        nc.scalar.activation(
                            ex16[:, sb, :], sc[:], EXP, scale=SCALING
                        )
                        fill(3)
                        for th in range(2):
                            rc = smallp.tile([128, 512], F32, tag="sm",
                                             name=f"rc{h}_{sb}_{th}")
                            if USE_RDR_SCORE_DR:
                                nc.tensor.matmul(
                                    rc[:],
                                    K8[rrow, :, ssl],
                                    Q8[rrow, :, th * 512 : (th + 1) * 512],
                                    start=True, stop=True,
                                    perf_mode=DRM,
                                    tile_position=(32 * hq, 0),
                                )
                            else:
                                for ab in range(2):
                                    nc.tensor.matmul(
                                        rc[:],
                                        K8[rrow, ab, ssl],
                                        Q8[rrow, ab, th * 512 : (th + 1) * 512],
                                        start=(ab == 0), stop=(ab == 1),
                                        tile_position=(32 * hq, 0),
                                    )
                            nc.vector.tensor_scalar(
                                ex8[:, sb, th * 512 : (th + 1) * 512].bitcast(U8),
                                rc[:],
                                SCH_A, SCH_B, MULT, ADD,
                            )
                            fill(1)
                        fill(1)

                    # queue this head's av work; emitted during later heads'
                    # scores loops via fill()
                    av_q.extend(av_steps(h, ex16, ex8))

                    if hp == 1:
                        while pump:
                            pump.pop(0)()
                        if nxt_g is not None:
                            Qg, Kg = nxt_g[0], nxt_g[1]
                            nxt_g = None
                        if hq == 3 and nxt_r is not None:
                            Q8, K8 = nxt_r[0], nxt_r[1]
                            nxt_r = None

                while av_q:
                    av_q.pop(0)()

            # ---------------- output projection -----------------------------
            with tc.tile_pool(name="ops", bufs=6, space="PSUM") as ops, \
                 tc.tile_pool(name="o16p", bufs=4) as o16p:
                # 4 half-chain accumulators live at once; prefix over pairs
                # 0..6 depends only on heads <= 13 so it overlaps the final
                # head's av/combine; the pair-7 step + drain go in wave 2.
                halves = [(j, th) for j in range(TB) for th in range(2)]
                pos = {}
                for w0 in range(0, 16, 6):
                    for j, th in halves[w0 : w0 + 6]:
                        tsl = slice(th * 512, (th + 1) * 512)
                        po = ops.tile([128, 512], F32, tag="po",
                                      name=f"po{j}_{th}")
                        pos[(j, th)] = po
                        for pr in range(NP - 1):
                            nc.tensor.matmul(
                                po[:],
                                wo_sb[:, j, pr, :],
                                attnT_tiles[pr][:, tsl],
                                start=(pr == 0),
                                stop=False,
                            )
                    for j, th in halves[w0 : w0 + 6]:
                        tsl = slice(th * 512, (th + 1) * 512)
                        po = pos.pop((j, th))
                        nc.tensor.matmul(
                            po[:],
                            wo_sb[:, j, NP - 1, :],
                            attnT_tiles[NP - 1][:, tsl],
                            start=False,
                            stop=True,
                        )
                        o16 = o16p.tile([128, 512], F16, tag="o16")
                        if th == 0:
                            nc.vector.tensor_scalar_add(
                                o16[:], po[:], bo_sb[:, j : j + 1]
                            )
                        else:
                            nc.scalar.activation(
                                o16[:], po[:],
                                mybir.ActivationFunctionType.Identity,
                                bias=bo_sb[:, j : j + 1],
                            )
                        nc.sync.dma_start(
                            outT[j * 128 : (j + 1) * 128, tsl], o16[:]
                        )

    nc.finalize()
    return nc


_NC_CACHE = {}


def get_nc():
    if "nc" not in _NC_CACHE:
        _NC_CACHE["nc"] = build_nc()
    return _NC_CACHE["nc"]


def _host_prep(hidden_states, reader_token, Wq, bq, Wk, bk, Wv, bv, Wo, bo,
               RWq, Rbq, RWk, Rbk, RWv, Rbv):
    f = np.float32
    np16 = mybir.dt.np(F16)
    np8 = mybir.dt.np(F8)
    hs = np.asarray(hidden_states, f)
    tok = np.asarray(reader_token).astype(np.int64)

    WqT = np.asarray(Wq, f).T  # [e, o]
    WkT = np.asarray(Wk, f).T
    WvT = np.asarray(Wv, f).T
    WoT = np.asarray(Wo, f).T
    bq = np.asarray(bq, f); bk = np.asarray(bk, f)
    bv = np.asarray(bv, f); bo_ = np.asarray(bo, f)
    Rbq = np.asarray(Rbq, f); Rbk = np.asarray(Rbk, f)

    # gen weights [NP, 128, 2, EO, 128]
    wg_arr = np.empty((NP, 128, 2, EO, 128), np16)
    for qk, WT in enumerate((WqT, WkT)):
        r = WT.reshape(EO, 128, NP, 128)  # (eo, p, pair, m)
        wg_arr[:, :, qk] = r.transpose(2, 1, 0, 3).astype(np16)
    # gen biases [128, 2, NP]
    gb = np.empty((128, 2, NP), f)
    for qk, bb in enumerate((bq, bk)):
        gb[:, qk, :] = bb.reshape(NP, 128).T

    # v-bias folds into output bias (probs rows sum to 0.5)
    bo_eff = bo_ + 0.5 * (np.asarray(Wo, f) @ bv)
    bo_t = np.ascontiguousarray(bo_eff.reshape(TB, 128).T)

    # wv [128, EO, E]
    wv_arr = np.ascontiguousarray(
        WvT.reshape(EO, 128, E).transpose(1, 0, 2)
    ).astype(np16)
    # wo [TB, 128, NP, 128]
    wo_arr = np.ascontiguousarray(
        WoT.reshape(NP, 128, TB, 128).transpose(2, 1, 0, 3)
    ).astype(np16)

    percore = {}
    in_maps = []
    for b in range(B):
        g = int(tok[b])
        if g not in percore:
            RWqT = np.asarray(RWq[g], f).T  # [e, o]
            RWkT = np.asarray(RWk[g], f).T
            w8_arr = np.empty((NQ, 128, 2, 2, EO, 128), np8)
            for qk, WT in enumerate((RWqT, RWkT)):
                # o = (quad*4 + m//32)*64 + ab*32 + m%32
                r = WT.reshape(EO, 128, NQ, 4, 2, 32)  # (eo,p,quad,hin,ab,dd)
                # -> (quad, p, qk, ab, eo, m=(hin,dd))
                w8_arr[:, :, qk] = r.transpose(2, 4, 1, 0, 3, 5).reshape(
                    NQ, 2, 128, EO, 128
                ).astype(np8).transpose(0, 2, 1, 3, 4).reshape(NQ, 128, 2, EO, 128)
            rb = np.empty((128, 2, NQ, 2), f)
            for qk, bb in enumerate((Rbq[g], Rbk[g])):
                # p = hin*32 + dd ; value = b[(quad*4+hin)*64 + ab*32 + dd]
                r = bb.reshape(NQ, 4, 2, 32)  # (quad, hin, ab, dd)
                rb[:, qk, :, :] = r.transpose(1, 3, 0, 2).reshape(128, NQ, 2)
            percore[g] = (w8_arr, rb)
        w8_arr, rb = percore[g]
        hsT = np.ascontiguousarray(hs[b].T)
        in_maps.append(
            {
                "hsT16": hsT.astype(np16),
                "hsT8": hsT.astype(np8),
                "wg": wg_arr,
                "w8": w8_arr,
                "wv": wv_arr,
                "wo": wo_arr,
                "gbias": gb,
                "rbias": rb,
                "bo": bo_t,
            }
        )
    return in_maps


def kernel(**inputs) -> np.ndarray:
    in_maps = _host_prep(**inputs)
    nc = get_nc()
    res = run_bass_kernel_spmd(nc, in_maps, list(range(B)))
    out = np.stack(
        [np.asarray(res.results[c]["outT"]).astype(np.float32).T for c in range(B)],
        axis=0,
    )
    return np.ascontiguousarray(out)

